# revision 9
# baseline (speedup 1.0000x reference)
# Multi-head attention (B=2, T=2048, D=1024, H=16) on 8 TRN2 NeuronCores.
#
# Sharding: tensor-parallel over heads. Each core owns 2 heads (a 128-wide
# slice of the hidden dim): it computes its q/k/v projection slice, full
# attention for its 4 (batch, head) pairs, and a partial output projection
# over its slice of the contraction. The 8 partial outputs are summed on the
# host (the TP all-reduce, done as part of unsharding), plus the output bias.
#
# Layouts (per core):
#   xT   [D=1024, B*T=4096]   x transposed so the contraction dim is on SBUF
#                             partitions for the projection matmuls.
#   qT/kT [128, 4096]         head-dim on partitions (2 heads stacked), token
#                             on free axis -> S^T tiles come out of the PE
#                             directly with softmax's reduction on the free
#                             axis of P^T's consumer.
#   v    [128tok, 32kt, 2h, 65]  natural [token, dim] layout per k-tile with a
#                             ones column appended: the ctx matmul then yields
#                             the softmax denominator for free in row 64.
#   ctxT [64, 2h, 4096]       per-head contraction layout for the output
#                             projection (K=64 accumulation over both heads).
import numpy as np

import concourse.bass as bass
import concourse.mybir as mybir
from concourse.bass_utils import run_bass_kernel_spmd
from concourse.masks import make_identity
from concourse.tile import TileContext

B, T, D, H = 2, 2048, 1024, 16
HD = D // H          # 64
NCORES = 8
BT = B * T           # 4096
E = D // NCORES      # 128 = per-core slice of hidden dim (2 heads)
HPC = E // HD        # 2 heads per core

F32 = mybir.dt.float32
F32R = mybir.dt.float32r
AF = mybir.ActivationFunctionType

# Matmul compute dtype: float32r streams fp32 data through the PE at full
# (bf16) rate with relaxed multiply precision. Set to F32 for exact-but-4x-
# slower matmuls.
MM_DT = F32R

TCH = 512            # token chunk for projections / q chunks
NTCH = BT // TCH     # 8
NKT = BT // 128      # 32 token tiles of 128
KTB = T // 128       # 16 k-tiles per batch


def build_nc():
    nc = bass.Bass()

    xT = nc.dram_tensor("xT", [D, BT], MM_DT, kind="ExternalInput")
    wqT = nc.dram_tensor("wqT", [D, E], MM_DT, kind="ExternalInput")
    wkT = nc.dram_tensor("wkT", [D, E], MM_DT, kind="ExternalInput")
    wvT = nc.dram_tensor("wvT", [D, E], MM_DT, kind="ExternalInput")
    bq = nc.dram_tensor("bq", [E, 1], F32, kind="ExternalInput")
    bk = nc.dram_tensor("bk", [E, 1], F32, kind="ExternalInput")
    bv = nc.dram_tensor("bv", [E, 1], F32, kind="ExternalInput")
    wo = nc.dram_tensor("wo", [HD, HPC, D], MM_DT, kind="ExternalInput")
    ones64 = nc.dram_tensor("ones64", [128, HD], MM_DT, kind="ExternalInput")
    out = nc.dram_tensor("out", [BT, D], F32, kind="ExternalOutput")

    with TileContext(nc) as tc:
        with (
            nc.allow_low_precision(reason="float32r is deliberate (matmul speed)"),
            tc.tile_pool(name="const", bufs=1) as cpool,
            tc.tile_pool(name="pers", bufs=1) as pers,
            tc.tile_pool(name="work", bufs=2) as work,
            tc.tile_pool(name="psum", bufs=2, space="PSUM") as psum,
        ):
            # ---- constants -------------------------------------------------
            wq_sb = cpool.tile([128, D // 128, E], MM_DT, name="wq_sb")
            wk_sb = cpool.tile([128, D // 128, E], MM_DT, name="wk_sb")
            wv_sb = cpool.tile([128, D // 128, E], MM_DT, name="wv_sb")
            nc.sync.dma_start(wq_sb, wqT.rearrange("(n p) m -> p n m", p=128))
            nc.sync.dma_start(wk_sb, wkT.rearrange("(n p) m -> p n m", p=128))
            nc.sync.dma_start(wv_sb, wvT.rearrange("(n p) m -> p n m", p=128))
            wo_sb = cpool.tile([HD, HPC, D], MM_DT, name="wo_sb")
            nc.sync.dma_start(wo_sb, wo[:, :, :])
            bq_sb = cpool.tile([E, 1], F32, name="bq_sb")
            bk_sb = cpool.tile([E, 1], F32, name="bk_sb")
            bv_sb = cpool.tile([E, 1], F32, name="bv_sb")
            nc.sync.dma_start(bq_sb, bq[:, :])
            nc.sync.dma_start(bk_sb, bk[:, :])
            nc.sync.dma_start(bv_sb, bv[:, :])
            ident = cpool.tile([128, 128], F32, name="ident")
            make_identity(nc, ident)
            ones_row = cpool.tile([128, HD], MM_DT, name="ones_row")
            nc.sync.dma_start(ones_row, ones64[:, :])

            # ---- persistent activations -----------------------------------
            qT = pers.tile([E, BT], MM_DT, name="qT")
            kT = pers.tile([E, BT], MM_DT, name="kT")
            v = pers.tile([128, NKT, HPC, HD + 1], MM_DT, name="v")
            ctxT = pers.tile([HD, HPC, BT], MM_DT, name="ctxT")
            nc.sync.dma_start(
                v[:, :, :, HD], ones64[:, : NKT * HPC]
            )

            # ---- phase A: QKV projections ---------------------------------
            for t in range(NTCH):
                cols = bass.ts(t, TCH)
                xt = work.tile([128, D // 128, TCH], MM_DT, name="xt", tag="xt", bufs=2)
                nc.sync.dma_start(
                    xt, xT[:, cols].rearrange("(n p) m -> p n m", p=128)
                )
                for w_sb, b_sb, dst in (
                    (wq_sb, bq_sb, qT),
                    (wk_sb, bk_sb, kT),
                    (wv_sb, bv_sb, None),
                ):
                    ps = psum.tile([128, TCH], F32, name="ps_mm", tag="mm", bufs=2)
                    for d in range(D // 128):
                        nc.tensor.matmul(
                            ps,
                            lhsT=w_sb[:, d, :],
                            rhs=xt[:, d, :],
                            start=(d == 0),
                            stop=(d == D // 128 - 1),
                        )
                    if dst is not None:
                        nc.scalar.activation(
                            dst[:, cols], ps, AF.Identity, bias=b_sb, scale=1.0
                        )
                    else:
                        vt = work.tile([128, TCH], F32, name="vt", tag="vt", bufs=2)
                        nc.scalar.activation(vt, ps, AF.Identity, bias=b_sb, scale=1.0)
                        # transpose v back to [token, dim] layout, 128 at a time
                        for i in range(TCH // 128):
                            kt_idx = t * (TCH // 128) + i
                            tp = psum.tile(
                                [128, 128], F32, name="tp", tag="tp", bufs=2
                            )
                            nc.tensor.transpose(tp, vt[:, bass.ts(i, 128)], ident)
                            for h in range(HPC):
                                nc.vector.tensor_copy(
                                    v[:, kt_idx, h, 0:HD], tp[:, bass.ts(h, HD)]
                                )

            # ---- phase B: attention (flash-style, per (batch, qchunk, head))
            for b in range(B):
                for qc in range(T // TCH):
                    q0 = b * T + qc * TCH
                    for h in range(HPC):
                        he = bass.ts(h, HD)
                        pts = []
                        for kt in range(KTB):
                            k0 = b * T + kt * 128
                            sp = psum.tile(
                                [128, TCH], F32, name="sp", tag="s", bufs=2
                            )
                            nc.tensor.matmul(
                                sp,
                                lhsT=kT[he, k0 : k0 + 128],
                                rhs=qT[he, q0 : q0 + TCH],
                                start=True,
                                stop=True,
                            )
                            pt = work.tile(
                                [128, TCH], MM_DT, name="pt", tag="pt", bufs=20
                            )
                            nc.scalar.activation(pt, sp, AF.Exp, scale=1.0 / 8.0)
                            pts.append(pt)
                        cp = psum.tile(
                            [HD + 1, TCH], F32, name="cp", tag="ctx", bufs=2
                        )
                        for kt in range(KTB):
                            nc.tensor.matmul(
                                cp,
                                lhsT=v[:, b * KTB + kt, h, :],
                                rhs=pts[kt],
                                start=(kt == 0),
                                stop=(kt == KTB - 1),
                            )
                        # normalize: ctxT = cp[0:64] * (1 / cp[64]) broadcast
                        cs = work.tile([HD + 1, TCH], MM_DT, name="cs", tag="cs", bufs=2)
                        nc.vector.tensor_copy(cs, cp)
                        nc.vector.reciprocal(
                            cs[HD : HD + 1, :], cs[HD : HD + 1, :]
                        )
                        # broadcast row 64 across 64 partitions with a K=1
                        # ones outer-product on the PE (engines can't shift
                        # partitions)
                        rb = psum.tile([HD, TCH], F32, name="rb", tag="tp", bufs=2)
                        nc.tensor.matmul(
                            rb,
                            lhsT=ones_row[HD : HD + 1, :],
                            rhs=cs[HD : HD + 1, :],
                            start=True,
                            stop=True,
                        )
                        nc.vector.tensor_tensor(
                            ctxT[:, h, q0 : q0 + TCH],
                            cs[0:HD, :],
                            rb,
                            op=mybir.AluOpType.mult,
                        )

            # ---- phase C: output projection (partial over this core's slice)
            for tt in range(NKT):
                trows = bass.ts(tt, 128)
                for nch in range(D // TCH):
                    po = psum.tile([128, TCH], F32, name="po", tag="mm", bufs=2)
                    for h in range(HPC):
                        nc.tensor.matmul(
                            po,
                            lhsT=ctxT[:, h, trows],
                            rhs=wo_sb[:, h, bass.ts(nch, TCH)],
                            start=(h == 0),
                            stop=(h == HPC - 1),
                        )
                    ob = work.tile([128, TCH], F32, name="ob", tag="ob", bufs=3)
                    nc.vector.tensor_copy(ob, po)
                    nc.sync.dma_start(out[trows, bass.ts(nch, TCH)], ob)

    _split_matmul_waits(nc)
    return nc


def _split_matmul_waits(nc):
    """This walrus allows only one sync wait per engine instruction (and none
    on fp32/f32r InstMatmult, whose embedded S3_LW carries the wait slot).
    Move excess waits onto InstEventSemaphore instructions (capacity 2)
    inserted just before the owner in the same engine stream — sequencer
    dispatch is in-order, so semantics are unchanged."""
    ctr = 0
    for f in nc.m.functions:
        for blk in f.blocks:
            out = []
            for inst in blk.instructions:
                si = inst.sync_info
                if (
                    si is not None
                    and not isinstance(inst, mybir.InstEventSemaphore)
                    and len(si.on_wait) > 1
                ):
                    waits = list(si.on_wait)
                    keep = [waits.pop(0)]
                    for i in range(0, len(waits), 2):
                        ev = mybir.InstEventSemaphore(name=f"I-exwait-{ctr}")
                        ctr += 1
                        ev.engine = inst.engine
                        ev.sync_info = mybir.SyncInfo(
                            on_wait=waits[i : i + 2], on_update=[]
                        )
                        nc.register_instruction(ev)
                        out.append(ev)
                    si.on_wait = keep
                out.append(inst)
            blk.instructions[:] = out


_CACHE = {}


def _get_nc():
    if "nc" not in _CACHE:
        _CACHE["nc"] = build_nc()
    return _CACHE["nc"]


def make_in_maps(x, w_qkv, b_qkv, w_out):
    x = np.ascontiguousarray(np.asarray(x, np.float32)).reshape(BT, D)
    w_qkv = np.asarray(w_qkv, np.float32)
    b_qkv = np.asarray(b_qkv, np.float32)
    w_out = np.asarray(w_out, np.float32)

    xT = np.ascontiguousarray(x.T)  # [D, BT]
    wq, wk, wv = w_qkv[0:D], w_qkv[D : 2 * D], w_qkv[2 * D : 3 * D]
    bqs, bks, bvs = b_qkv[0:D], b_qkv[D : 2 * D], b_qkv[2 * D : 3 * D]

    in_maps = []
    for c in range(NCORES):
        rs = slice(E * c, E * (c + 1))
        # wo_c[j, h, o] = w_out[o, E*c + h*HD + j]
        wo_c = np.ascontiguousarray(
            w_out[:, rs].T.reshape(HPC, HD, D).transpose(1, 0, 2)
        )
        in_maps.append(
            {
                "xT": xT,
                "wqT": np.ascontiguousarray(wq[rs].T),
                "wkT": np.ascontiguousarray(wk[rs].T),
                "wvT": np.ascontiguousarray(wv[rs].T),
                "bq": np.ascontiguousarray(bqs[rs])[:, None],
                "bk": np.ascontiguousarray(bks[rs])[:, None],
                "bv": np.ascontiguousarray(bvs[rs])[:, None],
                "wo": wo_c,
                "ones64": np.ones((128, HD), np.float32),
            }
        )
    return in_maps


def _combine(results, b_out):
    acc = results[0]["out"].copy()
    for r in results[1:]:
        acc += r["out"]
    acc += np.asarray(b_out, np.float32)[None, :]
    return acc.reshape(B, T, D)


def kernel(x, w_qkv, b_qkv, w_out, b_out):
    in_maps = make_in_maps(x, w_qkv, b_qkv, w_out)
    res = run_bass_kernel_spmd(_get_nc(), in_maps, core_ids=list(range(NCORES)))
    return _combine(res.results, b_out)


def kernel_traced(x, w_qkv, b_qkv, w_out, b_out):
    """Like kernel() but profiles the run; returns (output, exec_time_ns)."""
    in_maps = make_in_maps(x, w_qkv, b_qkv, w_out)
    res = run_bass_kernel_spmd(
        _get_nc(), in_maps, core_ids=list(range(NCORES)), trace=True
    )
    return _combine(res.results, b_out), res.exec_time_ns


# revision 12
# speedup vs baseline: 1.2079x; 1.2079x over previous
# Multi-head attention (B=2, T=2048, D=1024, H=16) on 8 TRN2 NeuronCores.
#
# Sharding: tensor-parallel over heads. Each core owns 2 heads (a 128-wide
# slice of the hidden dim): it computes its q/k/v projection slice, full
# attention for its 4 (batch, head) pairs, and a partial output projection
# over its slice of the contraction. The 8 partial outputs are summed on the
# host (the TP all-reduce, done as part of unsharding), plus the output bias.
#
# Layouts (per core):
#   xT   [D=1024, B*T=4096]   x transposed so the contraction dim is on SBUF
#                             partitions for the projection matmuls.
#   qT/kT [128, 4096]         head-dim on partitions (2 heads stacked), token
#                             on free axis -> S^T tiles come out of the PE
#                             directly with softmax's reduction on the free
#                             axis of P^T's consumer.
#   v    [128tok, 32kt, 2h, 65]  natural [token, dim] layout per k-tile with a
#                             ones column appended: the ctx matmul then yields
#                             the softmax denominator for free in row 64.
#   ctxT [64, 2h, 4096]       per-head contraction layout for the output
#                             projection (K=64 accumulation over both heads).
import numpy as np

import concourse.bass as bass
import concourse.mybir as mybir
from concourse.bass_utils import run_bass_kernel_spmd
from concourse.masks import make_identity
from concourse.tile import TileContext

B, T, D, H = 2, 2048, 1024, 16
HD = D // H          # 64
NCORES = 8
BT = B * T           # 4096
E = D // NCORES      # 128 = per-core slice of hidden dim (2 heads)
HPC = E // HD        # 2 heads per core

F32 = mybir.dt.float32
F32R = mybir.dt.float32r
AF = mybir.ActivationFunctionType

# Matmul compute dtype: float32r streams fp32 data through the PE at full
# (bf16) rate with relaxed multiply precision. Set to F32 for exact-but-4x-
# slower matmuls.
MM_DT = F32R

TCH = 512            # token chunk for projections / q chunks
NTCH = BT // TCH     # 8
NKT = BT // 128      # 32 token tiles of 128
KTB = T // 128       # 16 k-tiles per batch


def build_nc():
    nc = bass.Bass()

    xT = nc.dram_tensor("xT", [D, BT], MM_DT, kind="ExternalInput")
    wqT = nc.dram_tensor("wqT", [D, E], MM_DT, kind="ExternalInput")
    wkT = nc.dram_tensor("wkT", [D, E], MM_DT, kind="ExternalInput")
    wvT = nc.dram_tensor("wvT", [D, E], MM_DT, kind="ExternalInput")
    bq = nc.dram_tensor("bq", [E, 1], F32, kind="ExternalInput")
    bk = nc.dram_tensor("bk", [E, 1], F32, kind="ExternalInput")
    bv = nc.dram_tensor("bv", [E, 1], F32, kind="ExternalInput")
    wo = nc.dram_tensor("wo", [HD, HPC, D], MM_DT, kind="ExternalInput")
    ones64 = nc.dram_tensor("ones64", [128, HD], MM_DT, kind="ExternalInput")
    out = nc.dram_tensor("out", [BT, D], F32, kind="ExternalOutput")

    with TileContext(nc) as tc:
        with (
            nc.allow_low_precision(reason="float32r is deliberate (matmul speed)"),
            tc.tile_pool(name="const", bufs=1) as cpool,
            tc.tile_pool(name="pers", bufs=1) as pers,
            tc.tile_pool(name="work", bufs=2) as work,
            tc.tile_pool(name="psum", bufs=2, space="PSUM") as psum,
        ):
            # ---- constants -------------------------------------------------
            wq_sb = cpool.tile([128, D // 128, E], MM_DT, name="wq_sb")
            wk_sb = cpool.tile([128, D // 128, E], MM_DT, name="wk_sb")
            wv_sb = cpool.tile([128, D // 128, E], MM_DT, name="wv_sb")
            nc.sync.dma_start(wq_sb, wqT.rearrange("(n p) m -> p n m", p=128))
            nc.sync.dma_start(wk_sb, wkT.rearrange("(n p) m -> p n m", p=128))
            nc.sync.dma_start(wv_sb, wvT.rearrange("(n p) m -> p n m", p=128))
            wo_sb = cpool.tile([HD, HPC, D], MM_DT, name="wo_sb")
            nc.sync.dma_start(wo_sb, wo[:, :, :])
            bq_sb = cpool.tile([E, 1], F32, name="bq_sb")
            bk_sb = cpool.tile([E, 1], F32, name="bk_sb")
            bv_sb = cpool.tile([E, 1], F32, name="bv_sb")
            nc.sync.dma_start(bq_sb, bq[:, :])
            nc.sync.dma_start(bk_sb, bk[:, :])
            nc.sync.dma_start(bv_sb, bv[:, :])
            ident = cpool.tile([128, 128], F32, name="ident")
            make_identity(nc, ident)
            ones_row = cpool.tile([128, HD], MM_DT, name="ones_row")
            nc.sync.dma_start(ones_row, ones64[:, :])

            # ---- persistent activations -----------------------------------
            qT = pers.tile([E, BT], MM_DT, name="qT")
            kT = pers.tile([E, BT], MM_DT, name="kT")
            v = pers.tile([128, NKT, HPC, HD + 1], MM_DT, name="v")
            ctxT = pers.tile([HD, HPC, BT], MM_DT, name="ctxT")
            nc.sync.dma_start(
                v[:, :, :, HD], ones64[:, : NKT * HPC]
            )

            # ---- phase A: QKV projections ---------------------------------
            for t in range(NTCH):
                cols = bass.ts(t, TCH)
                xt = work.tile([128, D // 128, TCH], MM_DT, name="xt", tag="xt", bufs=2)
                nc.sync.dma_start(
                    xt, xT[:, cols].rearrange("(n p) m -> p n m", p=128)
                )
                for w_sb, b_sb, dst in (
                    (wq_sb, bq_sb, qT),
                    (wk_sb, bk_sb, kT),
                    (wv_sb, bv_sb, None),
                ):
                    ps = psum.tile([128, TCH], F32, name="ps_mm", tag="mm", bufs=2)
                    for d in range(D // 128):
                        nc.tensor.matmul(
                            ps,
                            lhsT=w_sb[:, d, :],
                            rhs=xt[:, d, :],
                            start=(d == 0),
                            stop=(d == D // 128 - 1),
                        )
                    if dst is not None:
                        nc.scalar.activation(
                            dst[:, cols], ps, AF.Identity, bias=b_sb, scale=1.0
                        )
                    else:
                        vt = work.tile([128, TCH], F32, name="vt", tag="vt", bufs=2)
                        nc.scalar.activation(vt, ps, AF.Identity, bias=b_sb, scale=1.0)
                        # transpose v back to [token, dim] layout, 128 at a time
                        for i in range(TCH // 128):
                            kt_idx = t * (TCH // 128) + i
                            tp = psum.tile(
                                [128, 128], F32, name="tp", tag="s", bufs=2
                            )
                            nc.tensor.transpose(tp, vt[:, bass.ts(i, 128)], ident)
                            for h in range(HPC):
                                nc.vector.tensor_copy(
                                    v[:, kt_idx, h, 0:HD], tp[:, bass.ts(h, HD)]
                                )

            # ---- phase B: attention (flash-style, per (batch, qchunk)) -----
            # Per k-tile, both heads' S^T matmuls are row-tiled (T0/T8) so
            # they run concurrently on the PE and T8's weight-load overlaps
            # T0's matmul; both land in one [128, 1024] PSUM tile so a single
            # Exp serves both heads. ctx matmuls are software-pipelined one
            # 4-k-tile block behind the S matmuls to keep the PE busy during
            # the exps without thrashing the PE tiling mode per k-tile.
            BLK = 4
            for b in range(B):
                for qc in range(T // TCH):
                    q0 = b * T + qc * TCH
                    cps = []
                    for h in range(HPC):
                        cp = psum.tile(
                            [HD + 1, TCH], F32, name=f"cp{h}", tag=f"ctx{h}", bufs=1
                        )
                        cps.append(cp)
                    pts = {}
                    for blk in range(KTB // BLK + 1):
                        if blk < KTB // BLK:
                            for kt in range(blk * BLK, (blk + 1) * BLK):
                                k0 = b * T + kt * 128
                                sp = psum.tile(
                                    [128, HPC * TCH], F32, name="sp", tag="s", bufs=2
                                )
                                for h in range(HPC):
                                    he = bass.ts(h, HD)
                                    nc.tensor.matmul(
                                        sp[:, bass.ts(h, TCH)],
                                        lhsT=kT[he, k0 : k0 + 128],
                                        rhs=qT[he, q0 : q0 + TCH],
                                        start=True,
                                        stop=True,
                                    )
                                pt = work.tile(
                                    [128, HPC * TCH], MM_DT, name="pt", tag="pt",
                                    bufs=13,
                                )
                                nc.scalar.activation(pt, sp, AF.Exp, scale=1.0 / 8.0)
                                pts[kt] = pt
                        if blk > 0:
                            for kt in range((blk - 1) * BLK, blk * BLK):
                                for h in range(HPC):
                                    nc.tensor.matmul(
                                        cps[h],
                                        lhsT=v[:, b * KTB + kt, h, :],
                                        rhs=pts[kt][:, bass.ts(h, TCH)],
                                        start=(kt == 0),
                                        stop=(kt == KTB - 1),
                                        skip_group_check=True,
                                    )
                    # normalize: ctxT = cp[0:64] * (1 / cp[64]) broadcast
                    for h in range(HPC):
                        cs = work.tile(
                            [HD + 1, TCH], MM_DT, name="cs", tag="cs", bufs=2
                        )
                        nc.vector.tensor_copy(cs, cps[h])
                        nc.vector.reciprocal(
                            cs[HD : HD + 1, :], cs[HD : HD + 1, :]
                        )
                        # broadcast the reciprocal row to all 64 ctx
                        # partitions with a K=1 ones outer-product on the PE
                        # (engines can't shift partitions)
                        rb = psum.tile([HD, TCH], F32, name="rb", tag="mm", bufs=2)
                        nc.tensor.matmul(
                            rb,
                            lhsT=ones_row[HD : HD + 1, :],
                            rhs=cs[HD : HD + 1, :],
                            start=True,
                            stop=True,
                        )
                        nc.vector.tensor_tensor(
                            ctxT[:, h, q0 : q0 + TCH],
                            cs[0:HD, :],
                            rb,
                            op=mybir.AluOpType.mult,
                        )

            # ---- phase C: output projection (partial over this core's slice)
            for tt in range(NKT):
                trows = bass.ts(tt, 128)
                for nch in range(D // TCH):
                    po = psum.tile([128, TCH], F32, name="po", tag="mm", bufs=2)
                    for h in range(HPC):
                        nc.tensor.matmul(
                            po,
                            lhsT=ctxT[:, h, trows],
                            rhs=wo_sb[:, h, bass.ts(nch, TCH)],
                            start=(h == 0),
                            stop=(h == HPC - 1),
                        )
                    ob = work.tile([128, TCH], F32, name="ob", tag="ob", bufs=3)
                    nc.vector.tensor_copy(ob, po)
                    nc.sync.dma_start(out[trows, bass.ts(nch, TCH)], ob)

    _split_matmul_waits(nc)
    return nc


def _split_matmul_waits(nc):
    """This walrus allows only one sync wait per engine instruction (and none
    on fp32/f32r InstMatmult, whose embedded S3_LW carries the wait slot).
    Move excess waits onto InstEventSemaphore instructions (capacity 2)
    inserted just before the owner in the same engine stream — sequencer
    dispatch is in-order, so semantics are unchanged."""
    ctr = 0
    for f in nc.m.functions:
        for blk in f.blocks:
            out = []
            for inst in blk.instructions:
                si = inst.sync_info
                if (
                    si is not None
                    and not isinstance(inst, mybir.InstEventSemaphore)
                    and len(si.on_wait) > 1
                ):
                    waits = list(si.on_wait)
                    keep = [waits.pop(0)]
                    for i in range(0, len(waits), 2):
                        ev = mybir.InstEventSemaphore(name=f"I-exwait-{ctr}")
                        ctr += 1
                        ev.engine = inst.engine
                        ev.sync_info = mybir.SyncInfo(
                            on_wait=waits[i : i + 2], on_update=[]
                        )
                        nc.register_instruction(ev)
                        out.append(ev)
                    si.on_wait = keep
                out.append(inst)
            blk.instructions[:] = out


_CACHE = {}


def _get_nc():
    if "nc" not in _CACHE:
        _CACHE["nc"] = build_nc()
    return _CACHE["nc"]


def make_in_maps(x, w_qkv, b_qkv, w_out):
    x = np.ascontiguousarray(np.asarray(x, np.float32)).reshape(BT, D)
    w_qkv = np.asarray(w_qkv, np.float32)
    b_qkv = np.asarray(b_qkv, np.float32)
    w_out = np.asarray(w_out, np.float32)

    xT = np.ascontiguousarray(x.T)  # [D, BT]
    wq, wk, wv = w_qkv[0:D], w_qkv[D : 2 * D], w_qkv[2 * D : 3 * D]
    bqs, bks, bvs = b_qkv[0:D], b_qkv[D : 2 * D], b_qkv[2 * D : 3 * D]

    in_maps = []
    for c in range(NCORES):
        rs = slice(E * c, E * (c + 1))
        # wo_c[j, h, o] = w_out[o, E*c + h*HD + j]
        wo_c = np.ascontiguousarray(
            w_out[:, rs].T.reshape(HPC, HD, D).transpose(1, 0, 2)
        )
        in_maps.append(
            {
                "xT": xT,
                "wqT": np.ascontiguousarray(wq[rs].T),
                "wkT": np.ascontiguousarray(wk[rs].T),
                "wvT": np.ascontiguousarray(wv[rs].T),
                "bq": np.ascontiguousarray(bqs[rs])[:, None],
                "bk": np.ascontiguousarray(bks[rs])[:, None],
                "bv": np.ascontiguousarray(bvs[rs])[:, None],
                "wo": wo_c,
                "ones64": np.ones((128, HD), np.float32),
            }
        )
    return in_maps


def _combine(results, b_out):
    acc = results[0]["out"].copy()
    for r in results[1:]:
        acc += r["out"]
    acc += np.asarray(b_out, np.float32)[None, :]
    return acc.reshape(B, T, D)


def kernel(x, w_qkv, b_qkv, w_out, b_out):
    in_maps = make_in_maps(x, w_qkv, b_qkv, w_out)
    res = run_bass_kernel_spmd(_get_nc(), in_maps, core_ids=list(range(NCORES)))
    return _combine(res.results, b_out)


def kernel_traced(x, w_qkv, b_qkv, w_out, b_out):
    """Like kernel() but profiles the run; returns (output, exec_time_ns)."""
    in_maps = make_in_maps(x, w_qkv, b_qkv, w_out)
    res = run_bass_kernel_spmd(
        _get_nc(), in_maps, core_ids=list(range(NCORES)), trace=True
    )
    return _combine(res.results, b_out), res.exec_time_ns
